# revision 6
# baseline (speedup 1.0000x reference)
"""Per-sample 21x21 depthwise conv over (32, 3, 512, 512), 8-way data-parallel
on Trainium2 via Bass/Tile.

Algorithm: for each kernel column j, the H-direction 1D conv is a banded
(Toeplitz) matmul on the TensorEngine: out[h, w] = sum_{i,j} k[i,j] *
xpad[h+i, w+j].  Output rows are tiled in chunks of 108 so the contraction
window (108 + 20 = 128) fits the PE's 128-partition contraction dim exactly,
giving ONE matmul [K=128, M=108, N=512] per (chunk, j) accumulated in PSUM
over the 21 j's.  The band matrix B_j[p, m] = k[p-m, j] is translation
invariant, so one [128, 21, 108] SBUF tile per image serves every chunk.
float32r operands stream at 1 cycle/row (vs 4 for fp32).

Sharding: batch 32 -> 4 samples per core, no communication.  The 12 images
per core run under a For_i_unrolled(unroll=3) dynamic loop to keep the
program small enough for fast walrus compilation.
"""
import sys

sys.path.insert(0, "/opt/trn_rl_repo")

import numpy as np
import concourse.bacc as bacc
import concourse.tile as tile
import concourse.mybir as mybir
from concourse.bass import ds
from concourse.bass_utils import run_bass_kernel_spmd

B, C, H, W = 32, 3, 512, 512
KS, PAD = 21, 10
NCORES = 8
BPC = B // NCORES  # samples per core
T = BPC * C  # images per core
HP = WP = H + 2 * PAD  # 532
MC = 108  # h_out rows per chunk (contraction window = MC + KS - 1 = 128)
CHUNKS = [(0, 108), (108, 108), (216, 108), (324, 108), (432, 80)]

_nc_cache: dict = {}


def _build_nc(reps: int = 1, unroll: int = 3):
    f32r = mybir.dt.float32r
    f32 = mybir.dt.float32
    nc = bacc.Bacc(
        "TRN2", target_bir_lowering=False, debug=False, enable_asserts=False
    )
    xpad_d = nc.dram_tensor("xpad", [T, HP, WP], f32r, kind="ExternalInput")
    wb_d = nc.dram_tensor("wb", [T, 128, KS, MC], f32r, kind="ExternalInput")
    y_d = nc.dram_tensor("y", [T, H, W], f32, kind="ExternalOutput")

    with tile.TileContext(nc) as tc:
        with (
            tc.tile_pool(name="wp", bufs=2) as wp,
            tc.tile_pool(name="xp", bufs=10) as xp,
            tc.tile_pool(name="op", bufs=5) as op,
            tc.tile_pool(name="ps", bufs=6, space="PSUM") as psp,
        ):

            def body(t):
                wt = wp.tile([128, KS, MC], f32r, tag="wt")
                nc.sync.dma_start(
                    wt[:], wb_d[ds(t, 1)].rearrange("o p k m -> (o p) k m")
                )
                xts, pss = [], []
                for c0, mc in CHUNKS:
                    kk = mc + KS - 1
                    xt = xp.tile([128, WP], f32r, tag="xt")
                    nc.sync.dma_start(
                        xt[0:kk, :],
                        xpad_d[ds(t, 1), c0 : c0 + kk, :].rearrange(
                            "o p w -> (o p) w"
                        ),
                    )
                    xts.append(xt)
                    ps = psp.tile([128, 512], f32, tag="ps")
                    pss.append(ps)
                # j-outer: 5 consecutive matmuls share the same stationary
                # weights wt[:, j, :], amortizing the fused LDWEIGHTS.
                for j in range(KS):
                    for ci, (c0, mc) in enumerate(CHUNKS):
                        kk = mc + KS - 1
                        nc.tensor.matmul(
                            pss[ci][0:mc, :],
                            wt[0:kk, j, 0:mc],
                            xts[ci][0:kk, j : j + W],
                            start=(j == 0),
                            stop=(j == KS - 1),
                        )
                for ci, (c0, mc) in enumerate(CHUNKS):
                    ot = op.tile([128, 512], f32, tag="ot")
                    nc.vector.tensor_copy(ot[0:mc, :], pss[ci][0:mc, :])
                    nc.scalar.dma_start(
                        y_d[ds(t, 1), c0 : c0 + mc, :].rearrange("o p w -> (o p) w"),
                        ot[0:mc, :],
                    )

            pe_hint = (mybir.EngineType.PE,)

            def unrolled_loop():
                with tc.For_i(0, T, unroll, hint_engines=pe_hint) as i:
                    for k in range(unroll):
                        body(i + k)

            if reps == 1:
                unrolled_loop()
            else:
                with tc.For_i(0, reps, 1, hint_engines=pe_hint):
                    unrolled_loop()

    nc.compile()
    return nc


def _host_prep(x: np.ndarray, kern: np.ndarray):
    """Pad image; build per-sample banded Toeplitz weights."""
    xpad = np.zeros((B, C, HP, WP), np.float32)
    xpad[:, :, PAD : PAD + H, PAD : PAD + W] = x

    # Wbs[s, p, j, m] = kern[s, 0, p - m, j] for 0 <= p - m < KS
    Wbs = np.zeros((B, 128, KS, MC), np.float32)
    m = np.arange(MC)
    i = np.arange(KS)
    P = np.broadcast_to(
        i[:, None, None] + m[None, None, :], (KS, KS, MC)
    )  # p = i + m
    J = np.broadcast_to(i[None, :, None], (KS, KS, MC))
    M = np.broadcast_to(m[None, None, :], (KS, KS, MC))
    I = np.broadcast_to(i[:, None, None], (KS, KS, MC))
    Wbs[:, P, J, M] = kern[:, 0][:, I, J]
    # replicate per channel -> per-image weights, matching image order (s, c)
    Wb = np.repeat(Wbs, C, axis=0)  # (B*C, 128, KS, MC)
    return xpad, Wb


def _execute(x: np.ndarray, kern: np.ndarray, reps: int = 1) -> np.ndarray:
    if reps not in _nc_cache:
        _nc_cache[reps] = _build_nc(reps)
    nc = _nc_cache[reps]
    xpad, Wb = _host_prep(np.asarray(x), np.asarray(kern))
    xpad = xpad.reshape(B * C, HP, WP)
    in_maps = [
        {
            "xpad": np.ascontiguousarray(xpad[i * T : (i + 1) * T]),
            "wb": np.ascontiguousarray(Wb[i * T : (i + 1) * T]),
        }
        for i in range(NCORES)
    ]
    res = run_bass_kernel_spmd(nc, in_maps, list(range(NCORES)))
    y = np.concatenate([res.results[i]["y"] for i in range(NCORES)], axis=0)
    return y.reshape(B, C, H, W)


def kernel(x: np.ndarray, kernel: np.ndarray) -> np.ndarray:
    return _execute(x, kernel, reps=1)


# revision 8
# speedup vs baseline: 1.1403x; 1.1403x over previous
"""Per-sample 21x21 depthwise conv over (32, 3, 512, 512), 8-way data-parallel
on Trainium2 via Bass/Tile.

Algorithm: for each kernel column j, the H-direction 1D conv is a banded
(Toeplitz) matmul on the TensorEngine: out[h, w] = sum_{i,j} k[i,j] *
xpad[h+i, w+j].  Output rows are tiled in chunks of 108 so the contraction
window (108 + 20 = 128) fits the PE's 128-partition contraction dim exactly,
giving ONE matmul [K=128, M=108, N=512] per (chunk, j) accumulated in PSUM
over the 21 j's.  The band matrix B_j[p, m] = k[p-m, j] is translation
invariant, so one [128, 21, 108] SBUF tile per image serves every chunk.
float32r operands stream at 1 cycle/row (vs 4 for fp32).

Sharding: batch 32 -> 4 samples per core, no communication.  The 12 images
per core run under a For_i_unrolled(unroll=3) dynamic loop to keep the
program small enough for fast walrus compilation.
"""
import sys

sys.path.insert(0, "/opt/trn_rl_repo")

import numpy as np
import concourse.bacc as bacc
import concourse.tile as tile
import concourse.mybir as mybir
from concourse.bass import ds
from concourse.bass_utils import run_bass_kernel_spmd

B, C, H, W = 32, 3, 512, 512
KS, PAD = 21, 10
NCORES = 8
BPC = B // NCORES  # samples per core
T = BPC * C  # images per core
HP = WP = H + 2 * PAD  # 532
MC = 108  # h_out rows per chunk (contraction window = MC + KS - 1 = 128)
CHUNKS = [(0, 108), (108, 108), (216, 108), (324, 108), (432, 80)]

_nc_cache: dict = {}


def _build_nc(reps: int = 1, unroll: int = 3):
    f32r = mybir.dt.float32r
    f32 = mybir.dt.float32
    nc = bacc.Bacc(
        "TRN2", target_bir_lowering=False, debug=False, enable_asserts=False
    )
    xpad_d = nc.dram_tensor("xpad", [T, HP, WP], f32r, kind="ExternalInput")
    wb_d = nc.dram_tensor("wb", [T, 128, KS, MC], f32r, kind="ExternalInput")
    y_d = nc.dram_tensor("y", [T, H, W], f32, kind="ExternalOutput")

    with tile.TileContext(nc) as tc:
        with (
            tc.tile_pool(name="wp", bufs=2) as wp,
            tc.tile_pool(name="xp", bufs=6) as xp,
            tc.tile_pool(name="op", bufs=4) as op,
            tc.tile_pool(name="ps", bufs=6, space="PSUM") as psp,
        ):

            def body(t):
                wt = wp.tile([128, KS, MC], f32r, tag="wt")
                nc.sync.dma_start(
                    wt[:], wb_d[ds(t, 1)].rearrange("o p k m -> (o p) k m")
                )
                for c0, mc in CHUNKS:
                    kk = mc + KS - 1
                    xt = xp.tile([128, WP], f32r, tag="xt")
                    nc.sync.dma_start(
                        xt[0:kk, :],
                        xpad_d[ds(t, 1), c0 : c0 + kk, :].rearrange(
                            "o p w -> (o p) w"
                        ),
                    )
                    ps = psp.tile([128, 512], f32, tag="ps")
                    for j in range(KS):
                        nc.tensor.matmul(
                            ps[0:mc, :],
                            wt[0:kk, j, 0:mc],
                            xt[0:kk, j : j + W],
                            start=(j == 0),
                            stop=(j == KS - 1),
                        )
                    ot = op.tile([128, 512], f32, tag="ot")
                    nc.vector.tensor_copy(ot[0:mc, :], ps[0:mc, :])
                    nc.scalar.dma_start(
                        y_d[ds(t, 1), c0 : c0 + mc, :].rearrange("o p w -> (o p) w"),
                        ot[0:mc, :],
                    )

            if reps == 1:
                tc.For_i_unrolled(0, T, 1, body, max_unroll=unroll)
            else:
                with tc.For_i(0, reps, 1):
                    tc.For_i_unrolled(0, T, 1, body, max_unroll=unroll)

    nc.compile()
    return nc


def _host_prep(x: np.ndarray, kern: np.ndarray):
    """Pad image; build per-sample banded Toeplitz weights."""
    xpad = np.zeros((B, C, HP, WP), np.float32)
    xpad[:, :, PAD : PAD + H, PAD : PAD + W] = x

    # Wbs[s, p, j, m] = kern[s, 0, p - m, j] for 0 <= p - m < KS
    Wbs = np.zeros((B, 128, KS, MC), np.float32)
    m = np.arange(MC)
    i = np.arange(KS)
    P = np.broadcast_to(
        i[:, None, None] + m[None, None, :], (KS, KS, MC)
    )  # p = i + m
    J = np.broadcast_to(i[None, :, None], (KS, KS, MC))
    M = np.broadcast_to(m[None, None, :], (KS, KS, MC))
    I = np.broadcast_to(i[:, None, None], (KS, KS, MC))
    Wbs[:, P, J, M] = kern[:, 0][:, I, J]
    # replicate per channel -> per-image weights, matching image order (s, c)
    Wb = np.repeat(Wbs, C, axis=0)  # (B*C, 128, KS, MC)
    return xpad, Wb


def _execute(x: np.ndarray, kern: np.ndarray, reps: int = 1) -> np.ndarray:
    if reps not in _nc_cache:
        _nc_cache[reps] = _build_nc(reps)
    nc = _nc_cache[reps]
    xpad, Wb = _host_prep(np.asarray(x), np.asarray(kern))
    xpad = xpad.reshape(B * C, HP, WP)
    in_maps = [
        {
            "xpad": np.ascontiguousarray(xpad[i * T : (i + 1) * T]),
            "wb": np.ascontiguousarray(Wb[i * T : (i + 1) * T]),
        }
        for i in range(NCORES)
    ]
    res = run_bass_kernel_spmd(nc, in_maps, list(range(NCORES)))
    y = np.concatenate([res.results[i]["y"] for i in range(NCORES)], axis=0)
    return y.reshape(B, C, H, W)


def kernel(x: np.ndarray, kernel: np.ndarray) -> np.ndarray:
    return _execute(x, kernel, reps=1)


# revision 9
# speedup vs baseline: 1.9968x; 1.7510x over previous
"""Per-sample 21x21 depthwise conv over (32, 3, 512, 512), 8-way data-parallel
on Trainium2 via Bass/Tile.

Algorithm: for each kernel column j, the H-direction 1D conv is a banded
(Toeplitz) matmul on the TensorEngine: out[h, w] = sum_{i,j} k[i,j] *
xpad[h+i, w+j].  Output rows are tiled in chunks of 108 so the contraction
window (108 + 20 = 128) fits the PE's 128-partition contraction dim exactly,
giving ONE matmul [K=128, M=108, N=512] per (chunk, j) accumulated in PSUM
over the 21 j's.  The band matrix B_j[p, m] = k[p-m, j] is translation
invariant, so one [128, 21, 108] SBUF tile per image serves every chunk.
float32r operands stream at 1 cycle/row (vs 4 for fp32).

Sharding: batch 32 -> 4 samples per core, no communication.  The 12 images
per core run under a For_i_unrolled(unroll=3) dynamic loop to keep the
program small enough for fast walrus compilation.
"""
import sys

sys.path.insert(0, "/opt/trn_rl_repo")

import numpy as np
import concourse.bacc as bacc
import concourse.tile as tile
import concourse.mybir as mybir
from concourse.bass import ds
from concourse.bass_utils import run_bass_kernel_spmd

B, C, H, W = 32, 3, 512, 512
KS, PAD = 21, 10
NCORES = 8
BPC = B // NCORES  # samples per core
T = BPC * C  # images per core
HP = WP = H + 2 * PAD  # 532
MC = 108  # h_out rows per chunk (contraction window = MC + KS - 1 = 128)
CHUNKS = [(0, 108), (108, 108), (216, 108), (324, 108), (432, 80)]

_nc_cache: dict = {}


def _build_nc(reps: int = 1, unroll: int = 3):
    f32r = mybir.dt.float32r
    f32 = mybir.dt.float32
    nc = bacc.Bacc(
        "TRN2", target_bir_lowering=False, debug=False, enable_asserts=False
    )
    xpad_d = nc.dram_tensor("xpad", [T, HP, WP], f32r, kind="ExternalInput")
    wb_d = nc.dram_tensor("wb", [T, 128, KS, MC], f32r, kind="ExternalInput")
    y_d = nc.dram_tensor("y", [T, H, W], f32, kind="ExternalOutput")

    with tile.TileContext(nc) as tc:
        with (
            tc.tile_pool(name="wp", bufs=2) as wp,
            tc.tile_pool(name="xp", bufs=6) as xp,
            tc.tile_pool(name="op", bufs=4) as op,
            tc.tile_pool(name="ps", bufs=6, space="PSUM") as psp,
        ):

            def body(t):
                wt = wp.tile([128, KS, MC], f32r, tag="wt")
                nc.sync.dma_start(
                    wt[:], wb_d[ds(t, 1)].rearrange("o p k m -> (o p) k m")
                )
                for c0, mc in CHUNKS:
                    kk = mc + KS - 1
                    xt = xp.tile([128, WP], f32r, tag="xt")
                    nc.sync.dma_start(
                        xt[0:kk, :],
                        xpad_d[ds(t, 1), c0 : c0 + kk, :].rearrange(
                            "o p w -> (o p) w"
                        ),
                    )
                    ps = psp.tile([128, 512], f32, tag="ps")
                    for j in range(KS):
                        nc.tensor.matmul(
                            ps[0:mc, :],
                            wt[0:kk, j, 0:mc],
                            xt[0:kk, j : j + W],
                            start=(j == 0),
                            stop=(j == KS - 1),
                        )
                    ot = op.tile([128, 512], f32, tag="ot")
                    nc.vector.tensor_copy(ot[0:mc, :], ps[0:mc, :])
                    nc.scalar.dma_start(
                        y_d[ds(t, 1), c0 : c0 + mc, :].rearrange("o p w -> (o p) w"),
                        ot[0:mc, :],
                    )

            # Manual unroll under For_i so we can pass hint_engines: the body
            # exceeds 256 PE instructions (one IRAM block), so without branch
            # prefetch hints every back-edge stalls ~3-4us on the I$ fetch.
            pe_hint = (mybir.EngineType.PE,)

            def unrolled_loop():
                with tc.For_i(0, T, unroll, hint_engines=pe_hint) as i:
                    for k in range(unroll):
                        body(i + k)

            if reps == 1:
                unrolled_loop()
            else:
                with tc.For_i(0, reps, 1, hint_engines=pe_hint):
                    unrolled_loop()

    nc.compile()
    return nc


def _host_prep(x: np.ndarray, kern: np.ndarray):
    """Pad image; build per-sample banded Toeplitz weights."""
    xpad = np.zeros((B, C, HP, WP), np.float32)
    xpad[:, :, PAD : PAD + H, PAD : PAD + W] = x

    # Wbs[s, p, j, m] = kern[s, 0, p - m, j] for 0 <= p - m < KS
    Wbs = np.zeros((B, 128, KS, MC), np.float32)
    m = np.arange(MC)
    i = np.arange(KS)
    P = np.broadcast_to(
        i[:, None, None] + m[None, None, :], (KS, KS, MC)
    )  # p = i + m
    J = np.broadcast_to(i[None, :, None], (KS, KS, MC))
    M = np.broadcast_to(m[None, None, :], (KS, KS, MC))
    I = np.broadcast_to(i[:, None, None], (KS, KS, MC))
    Wbs[:, P, J, M] = kern[:, 0][:, I, J]
    # replicate per channel -> per-image weights, matching image order (s, c)
    Wb = np.repeat(Wbs, C, axis=0)  # (B*C, 128, KS, MC)
    return xpad, Wb


def _execute(x: np.ndarray, kern: np.ndarray, reps: int = 1) -> np.ndarray:
    if reps not in _nc_cache:
        _nc_cache[reps] = _build_nc(reps)
    nc = _nc_cache[reps]
    xpad, Wb = _host_prep(np.asarray(x), np.asarray(kern))
    xpad = xpad.reshape(B * C, HP, WP)
    in_maps = [
        {
            "xpad": np.ascontiguousarray(xpad[i * T : (i + 1) * T]),
            "wb": np.ascontiguousarray(Wb[i * T : (i + 1) * T]),
        }
        for i in range(NCORES)
    ]
    res = run_bass_kernel_spmd(nc, in_maps, list(range(NCORES)))
    y = np.concatenate([res.results[i]["y"] for i in range(NCORES)], axis=0)
    return y.reshape(B, C, H, W)


def kernel(x: np.ndarray, kernel: np.ndarray) -> np.ndarray:
    return _execute(x, kernel, reps=1)
